# revision 22
# baseline (speedup 1.0000x reference)
"""Cross-attention Trainium2 kernel, batch-parallel across 8 NeuronCores.

Per core: one batch element. LN(x) -> qT via transposed projection,
LN(ctx) -> kT / v, transposed-layout attention (keys on partitions).
Softmax normalization: denominators are broadcast across 64 partitions by
the PE itself (ones-matmul, column-tiled per head pair), reciprocal via the
fast DVE approximation, one tensor_mul evict per head pair. The null
key/value token is folded in with constant selector matmuls (no K=1 ops,
no DMA round-trips). The out projection of chunk c-1 is interleaved into
the sim/exp phase of chunk c so the PE keeps running while the scalar
engine exponentiates. All matmuls bf16 with f32 PSUM accumulation; LN
scale factors and num_heads**-0.5 are folded into the weights on the host.
"""
import numpy as np
import ml_dtypes

import concourse.bass as bass
from concourse import bacc
import concourse.mybir as mybir
import concourse.tile as tile
from concourse.bass_utils import run_bass_kernel_spmd
from concourse.masks import make_identity

BF = mybir.dt.bfloat16
F32 = mybir.dt.float32
NPBF = ml_dtypes.bfloat16

B, N_FULL, M, DIM = 8, 4096, 256, 1024
H, D = 16, 64
INNER = H * D
EPS = 1e-6
SCALE = H ** -0.5

_cache = {}


def _build(n_rows, apply_mask, trivial_lno):
    nchunks = n_rows // 512
    nc = bacc.Bacc(None, target_bir_lowering=False)
    x_d = nc.dram_tensor("x", [n_rows, DIM], BF, kind="ExternalInput")
    ctx_d = nc.dram_tensor("ctx", [M, DIM], BF, kind="ExternalInput")
    wq_d = nc.dram_tensor("wq", [DIM, INNER], BF, kind="ExternalInput")
    wk_d = nc.dram_tensor("wk", [DIM, INNER], BF, kind="ExternalInput")
    wv_d = nc.dram_tensor("wv", [DIM, INNER], BF, kind="ExternalInput")
    wo_d = nc.dram_tensor("wo", [INNER, DIM], BF, kind="ExternalInput")
    nullkblk_d = nc.dram_tensor("nullkblk", [128, 8, 16], BF, kind="ExternalInput")
    sel1_d = nc.dram_tensor("sel1", [128, INNER], BF, kind="ExternalInput")
    nvsel_d = nc.dram_tensor("nvsel", [128, INNER], BF, kind="ExternalInput")
    maskb_d = nc.dram_tensor("maskb", [128, 2], F32, kind="ExternalInput")
    lnos_d = nc.dram_tensor("lnos", [1, DIM], F32, kind="ExternalInput")
    lnob_d = nc.dram_tensor("lnob", [1, DIM], F32, kind="ExternalInput")
    out_d = nc.dram_tensor("out", [n_rows, DIM], F32, kind="ExternalOutput")

    with tile.TileContext(nc) as tc:
        with tc.tile_pool(name="const", bufs=1) as cst, \
             tc.tile_pool(name="sbw", bufs=1) as sbw, \
             tc.tile_pool(name="sbx", bufs=2) as sbx, \
             tc.tile_pool(name="sbq", bufs=1) as sbq, \
             tc.tile_pool(name="sbs", bufs=2) as sbs, \
             tc.tile_pool(name="sbo", bufs=2) as sbo, \
             tc.tile_pool(name="sbo1", bufs=1) as sbo1, \
             tc.tile_pool(name="pproj", bufs=2, space="PSUM") as pproj, \
             tc.tile_pool(name="psim", bufs=2, space="PSUM") as psim, \
             tc.tile_pool(name="pnul", bufs=1, space="PSUM") as pnul, \
             tc.tile_pool(name="pden", bufs=2, space="PSUM") as pden, \
             tc.tile_pool(name="pout", bufs=1, space="PSUM") as pout:

            ident = cst.tile([128, 128], BF, tag="ident")
            make_identity(nc, ident)
            epst = cst.tile([128, 1], F32, tag="epst")
            nc.vector.memset(epst, EPS)
            ones_bc = cst.tile([128, 64], BF, tag="ones_bc")
            nc.vector.memset(ones_bc, 1.0)
            nullkblk = cst.tile([128, 8, 16], BF, tag="nullkblk")
            nc.sync.dma_start(out=nullkblk, in_=nullkblk_d[:, :, :])
            sel1 = cst.tile([128, INNER], BF, tag="sel1")
            nc.sync.dma_start(out=sel1, in_=sel1_d[:, :])
            nvsel = cst.tile([128, INNER], BF, tag="nvsel")
            nc.sync.dma_start(out=nvsel, in_=nvsel_d[:, :])
            enullp = cst.tile([128, 512], BF, tag="enullp")
            nc.vector.memset(enullp, 0.0)
            if apply_mask:
                maskb = cst.tile([128, 2], F32, tag="maskb")
                nc.sync.dma_start(out=maskb, in_=maskb_d[:, :])
            if not trivial_lno:
                lnos = cst.tile([128, DIM], F32, tag="lnos")
                lnob = cst.tile([128, DIM], F32, tag="lnob")
                nc.sync.dma_start(out=lnos, in_=bass.AP(
                    tensor=lnos_d, offset=0, ap=[[0, 128], [1, DIM]]))
                nc.sync.dma_start(out=lnob, in_=bass.AP(
                    tensor=lnob_d, offset=0, ap=[[0, 128], [1, DIM]]))

            # persistent weights / context tensors
            wq = sbw.tile([128, 8, INNER], BF, tag="wq")
            wo = sbw.tile([128, 8, DIM], BF, tag="wo")
            for j in range(8):
                nc.sync.dma_start(out=wq[:, j], in_=wq_d.rearrange("(j p) i -> p j i", p=128)[:, j])
                nc.sync.dma_start(out=wo[:, j], in_=wo_d.rearrange("(j p) i -> p j i", p=128)[:, j])
            kT = sbw.tile([128, 8, 256], BF, tag="kT")
            v_sb = sbw.tile([128, 2, 16, 64], BF, tag="v_sb")
            eT_all = sbw.tile([128, 16, 2, 512], BF, tag="eT_all")

            def ln_rows(dst_bf, src_tile, rcount, tag):
                """LayerNorm rows of src_tile [128, rcount, DIM] -> dst_bf (bf16).
                Batched stats; rstd = 1/sqrt(var+eps) via Sqrt table + DVE
                reciprocal. Returns nothing."""
                stats = sbs.tile([128, rcount, 2, 6], F32, tag=f"st{tag}")
                mv = sbs.tile([128, rcount, 2], F32, tag=f"mv{tag}")
                for r in range(rcount):
                    nc.vector.bn_stats(stats[:, r, 0, :], src_tile[:, r, 0:512])
                    nc.vector.bn_stats(stats[:, r, 1, :], src_tile[:, r, 512:1024])
                    nc.vector.bn_aggr(mv[:, r, :], stats[:, r])
                sd = sbs.tile([128, rcount, 1], F32, tag=f"sd{tag}")
                nc.scalar.activation(
                    sd, mv[:, :, 1:2],
                    mybir.ActivationFunctionType.Sqrt, bias=epst, scale=1.0)
                rstd = sbs.tile([128, rcount, 1], F32, tag=f"rs{tag}")
                nc.vector.reciprocal(rstd, sd)
                for r in range(rcount):
                    nc.vector.tensor_scalar(
                        out=dst_bf[:, r, :], in0=src_tile[:, r, :],
                        scalar1=mv[:, r, 0:1], scalar2=rstd[:, r],
                        op0=mybir.AluOpType.subtract, op1=mybir.AluOpType.mult)

            # ---------------- context phase ----------------
            with tc.tile_pool(name="ctxw", bufs=1) as ctxw:
                wk = ctxw.tile([128, 8, INNER], BF, tag="wkv")
                for j in range(8):
                    nc.sync.dma_start(out=wk[:, j], in_=wk_d.rearrange("(j p) i -> p j i", p=128)[:, j])
                call = ctxw.tile([128, 2, DIM], BF, tag="call")
                nc.sync.dma_start(out=call, in_=ctx_d.rearrange("(r p) d -> p r d", p=128))
                cn = ctxw.tile([128, 2, DIM], BF, tag="cn")
                ln_rows(cn, call, 2, "c")
                cnT = ctxw.tile([128, 8, 256], BF, tag="cnT")
                for mm in range(2):
                    for g in range(2):
                        ptr = pproj.tile([128, 512], BF, tag="proj")
                        for b4 in range(4):
                            jj = g * 4 + b4
                            nc.tensor.transpose(ptr[:, 128 * b4:128 * (b4 + 1)],
                                                cn[:, mm, 128 * jj:128 * (jj + 1)], ident)
                        nc.vector.tensor_copy(
                            cnT[:, g * 4:(g + 1) * 4, 128 * mm:128 * (mm + 1)],
                            ptr.rearrange("p (a b) -> p a b", a=4))
                for i in range(8):
                    pk = pproj.tile([128, 512], F32, tag="proj")
                    for j in range(8):
                        nc.tensor.matmul(pk[:, 0:256], wk[:, j, 128 * i:128 * (i + 1)],
                                         cnT[:, j, :], start=(j == 0), stop=(j == 7))
                    nc.vector.tensor_copy(kT[:, i, :], pk[:, 0:256])
                wv = ctxw.tile([128, 8, INNER], BF, tag="wkv")
                for j in range(8):
                    nc.sync.dma_start(out=wv[:, j], in_=wv_d.rearrange("(j p) i -> p j i", p=128)[:, j])
                for mm in range(2):
                    for nh in range(2):
                        pv = pproj.tile([128, 512], F32, tag="proj")
                        for j in range(8):
                            nc.tensor.matmul(pv, cnT[:, j, 128 * mm:128 * (mm + 1)],
                                             wv[:, j, 512 * nh:512 * (nh + 1)],
                                             start=(j == 0), stop=(j == 7))
                        nc.vector.tensor_copy(
                            v_sb[:, mm, 8 * nh:8 * (nh + 1), :],
                            pv.rearrange("p (h d) -> p h d", h=8))

            # ---------------- main loop ----------------
            def load_ln_x(c):
                xall = sbx.tile([128, 4, DIM], BF, tag="xall")
                nc.sync.dma_start(
                    out=xall,
                    in_=x_d.rearrange("(c r p) d -> c p r d", c=nchunks, p=128)[c])
                xn = sbx.tile([128, 4, DIM], BF, tag="xn")
                ln_rows(xn, xall, 4, "x")
                return xn

            def transposes_q(xn):
                xnT = sbq.tile([128, 8, 512], BF, tag="xnT")
                for r in range(4):
                    for g in range(2):
                        ptr = pproj.tile([128, 512], BF, tag="proj")
                        for b4 in range(4):
                            jj = g * 4 + b4
                            nc.tensor.transpose(ptr[:, 128 * b4:128 * (b4 + 1)],
                                                xn[:, r, 128 * jj:128 * (jj + 1)], ident)
                        nc.vector.tensor_copy(
                            xnT[:, g * 4:(g + 1) * 4, 128 * r:128 * (r + 1)],
                            ptr.rearrange("p (a b) -> p a b", a=4))
                qT = sbq.tile([128, 8, 512], BF, tag="qT")
                for i in range(8):
                    pq = pproj.tile([128, 512], F32, tag="proj")
                    for j in range(8):
                        nc.tensor.matmul(pq, wq[:, j, 128 * i:128 * (i + 1)], xnT[:, j, :],
                                         start=(j == 0), stop=(j == 7))
                    nc.vector.tensor_copy(qT[:, i, :], pq)
                # null-key scores for all heads -> exp into enullp rows 0-15
                pn = pnul.tile([16, 512], F32, tag="pnull")
                for j in range(8):
                    nc.tensor.matmul(pn, nullkblk[:, j, :], qT[:, j, :],
                                     start=(j == 0), stop=(j == 7))
                nc.scalar.activation(enullp[0:16, :], pn,
                                     mybir.ActivationFunctionType.Exp)
                return qT

            def sim_unit(qT, h, kt):
                """sim for (head h, key tile kt): one N=512 matmul + exp evict."""
                j, po = h // 2, 64 * (h % 2)
                ps = psim.tile([128, 512], F32, tag="sim")
                nc.tensor.matmul(ps, kT[po:po + 64, j, 128 * kt:128 * (kt + 1)],
                                 qT[po:po + 64, j, :], start=True, stop=True)
                if apply_mask:
                    nc.scalar.activation(eT_all[:, h, kt, :], ps,
                                         mybir.ActivationFunctionType.Exp,
                                         bias=maskb[:, kt:kt + 1])
                else:
                    nc.scalar.activation(eT_all[:, h, kt, :], ps,
                                         mybir.ActivationFunctionType.Exp)

            def attn_pair(p, outT, mid=None):
                """denominator-broadcast + attention-out for head pair p.
                The two heads go to PSUM partition halves 0-63 / 64-127 via
                column tiling, so their matmuls run concurrently. `mid` is an
                optional callback issued between the den and po groups (used
                to interleave next-group sim matmuls)."""
                h0, h1 = 2 * p, 2 * p + 1
                den = pden.tile([128, 512], F32, tag="den")
                po_ps = pout.tile([128, 512], F32, tag="po")
                halves = ((0, h0), (64, h1))
                for kt in range(2):
                    for base, h in halves:
                        nc.tensor.matmul(den[base:base + 64, :], ones_bc,
                                         eT_all[:, h, kt, :],
                                         start=(kt == 0), stop=False)
                for base, h in halves:
                    nc.tensor.matmul(den[base:base + 64, :],
                                     sel1[:, 64 * h:64 * (h + 1)],
                                     enullp, start=False, stop=True)
                recb = sbs.tile([128, 512], F32, tag="recb")
                nc.vector.reciprocal_approx_fast(out=recb, in_=den)
                if mid is not None:
                    mid()
                for kt in range(2):
                    for base, h in halves:
                        nc.tensor.matmul(po_ps[base:base + 64, :],
                                         v_sb[:, kt, h, :],
                                         eT_all[:, h, kt, :],
                                         start=(kt == 0), stop=False)
                for base, h in halves:
                    nc.tensor.matmul(po_ps[base:base + 64, :],
                                     nvsel[:, 64 * h:64 * (h + 1)],
                                     enullp, start=False, stop=True)
                nc.vector.tensor_mul(outT[:, p, :], po_ps, recb)

            def outproj_m(outT, m, pfs):
                """out projection for row quarter m of the previous chunk."""
                for nh in range(2):
                    pf = pproj.tile([128, 512], F32, tag="proj")
                    for j in range(8):
                        nc.tensor.matmul(pf, outT[:, j, 128 * m:128 * (m + 1)],
                                         wo[:, j, 512 * nh:512 * (nh + 1)],
                                         start=(j == 0), stop=(j == 7))
                    pfs.append(pf)

            def outproj_ln_tail(cp, pf_list):
                """LN stats + apply + store for the 8 psum tiles of chunk cp."""
                oraw = sbo1.tile([128, 4, DIM], F32, tag="oraw")
                stats = sbs.tile([128, 4, 2, 6], F32, tag="sto")
                mvo = sbs.tile([128, 4, 2], F32, tag="mvo")
                for m in range(4):
                    for nh in range(2):
                        pf = pf_list[2 * m + nh]
                        nc.vector.bn_stats(stats[:, m, nh, :], pf)
                        nc.scalar.activation(oraw[:, m, 512 * nh:512 * (nh + 1)], pf,
                                             mybir.ActivationFunctionType.Copy)
                    nc.vector.bn_aggr(mvo[:, m, :], stats[:, m])
                sdo = sbs.tile([128, 4, 1], F32, tag="sdo")
                nc.scalar.activation(
                    sdo, mvo[:, :, 1:2],
                    mybir.ActivationFunctionType.Sqrt, bias=epst, scale=1.0)
                rstdo = sbs.tile([128, 4, 1], F32, tag="rso")
                nc.vector.reciprocal(rstdo, sdo)
                for m in range(4):
                    orow = sbo.tile([128, DIM], F32, tag="orow")
                    nc.vector.tensor_scalar(
                        out=orow, in0=oraw[:, m, :],
                        scalar1=mvo[:, m, 0:1], scalar2=rstdo[:, m],
                        op0=mybir.AluOpType.subtract, op1=mybir.AluOpType.mult)
                    if not trivial_lno:
                        nc.vector.tensor_mul(orow, orow, lnos)
                        nc.vector.tensor_add(orow, orow, lnob)
                    nc.sync.dma_start(
                        out=out_d[cp * 512 + 128 * m: cp * 512 + 128 * (m + 1), :],
                        in_=orow)

            prev_outT = None
            prev_c = None
            xn_cur = load_ln_x(0)
            for c in range(nchunks):
                qT = transposes_q(xn_cur)
                if c + 1 < nchunks:
                    xn_cur = load_ln_x(c + 1)
                outT = sbo.tile([128, 8, 512], BF, tag="outT")
                pf_list = []
                # interleave: sim pairs 0-3 with prev-chunk out projection
                for k in range(4):
                    sim_unit(qT, 2 * k, 0)
                    sim_unit(qT, 2 * k + 1, 0)
                    if prev_outT is not None:
                        outproj_m(prev_outT, k, pf_list)
                    sim_unit(qT, 2 * k, 1)
                    sim_unit(qT, 2 * k + 1, 1)
                if prev_outT is not None:
                    outproj_ln_tail(prev_c, pf_list)
                # attention-out pairs, interleaved with remaining sims
                for p in range(8):
                    if p < 4:
                        hh = 2 * (p + 4)
                        sim_unit(qT, hh, 0)
                        sim_unit(qT, hh + 1, 0)
                        mid = (lambda hh=hh: (sim_unit(qT, hh, 1),
                                              sim_unit(qT, hh + 1, 1)))
                    else:
                        mid = None
                    attn_pair(p, outT, mid)
                prev_outT, prev_c = outT, c

            # tail: out projection of the last chunk
            pf_list = []
            for k in range(4):
                outproj_m(prev_outT, k, pf_list)
            outproj_ln_tail(prev_c, pf_list)

    nc.compile()
    return nc


def _get_nc(n_rows, apply_mask, trivial_lno):
    key = (n_rows, apply_mask, trivial_lno)
    if key not in _cache:
        _cache[key] = _build(n_rows, apply_mask, trivial_lno)
    return _cache[key]


def kernel(x, context, mask, ln1_s, ln1_b, lnc_s, lnc_b, Wq, Wkv, null_kv, Wo,
           lno_s, lno_b, _n_rows=None, _return_bkr=False, _trace=False):
    x = np.asarray(x); context = np.asarray(context); mask = np.asarray(mask)
    n_rows = _n_rows or x.shape[1]
    Wq = np.asarray(Wq, np.float32); Wkv = np.asarray(Wkv, np.float32)
    Wo = np.asarray(Wo, np.float32); null_kv = np.asarray(null_kv, np.float32)
    ln1_s = np.asarray(ln1_s, np.float32); ln1_b = np.asarray(ln1_b, np.float32)
    lnc_s = np.asarray(lnc_s, np.float32); lnc_b = np.asarray(lnc_b, np.float32)
    lno_s = np.asarray(lno_s, np.float32); lno_b = np.asarray(lno_b, np.float32)

    Wk, Wv = Wkv[:, :INNER], Wkv[:, INNER:]
    wq_eff = (ln1_s[:, None] * Wq * SCALE).astype(NPBF)
    wk_eff = (lnc_s[:, None] * Wk).astype(NPBF)
    wv_eff = (lnc_s[:, None] * Wv).astype(NPBF)
    bq = (ln1_b @ Wq) * SCALE
    bk = lnc_b @ Wk
    bv = lnc_b @ Wv
    assert np.abs(bq).max() == 0 and np.abs(bk).max() == 0 and np.abs(bv).max() == 0, \
        "nonzero LN biases not supported by this build"
    wo_bf = Wo.astype(NPBF)
    nullkblk = np.zeros((128, 8, 16), np.float32)
    for j in range(8):
        nullkblk[0:64, j, 2 * j] = null_kv[0]
        nullkblk[64:128, j, 2 * j + 1] = null_kv[0]
    nullkblk = nullkblk.astype(NPBF)
    sel1 = np.zeros((128, INNER), np.float32)
    nvsel = np.zeros((128, INNER), np.float32)
    for h in range(H):
        sel1[h, 64 * h:64 * (h + 1)] = 1.0
        nvsel[h, 64 * h:64 * (h + 1)] = null_kv[1]
    sel1 = sel1.astype(NPBF)
    nvsel = nvsel.astype(NPBF)

    trivial_lno = bool(np.all(lno_s == 1.0) and np.all(lno_b == 0.0))
    apply_mask = not bool(mask.all())
    nc = _get_nc(n_rows, apply_mask, trivial_lno)

    in_maps = []
    for core in range(B):
        mb = np.zeros((128, 2), np.float32)
        if apply_mask:
            mb = np.where(mask[core].reshape(2, 128).T, 0.0, -1e30).astype(np.float32)
        in_maps.append({
            "x": x[core, :n_rows].astype(NPBF),
            "ctx": context[core].astype(NPBF),
            "wq": wq_eff, "wk": wk_eff, "wv": wv_eff, "wo": wo_bf,
            "nullkblk": nullkblk, "sel1": sel1, "nvsel": nvsel,
            "maskb": mb,
            "lnos": lno_s.reshape(1, DIM), "lnob": lno_b.reshape(1, DIM),
        })
    bkr = run_bass_kernel_spmd(nc, in_maps, core_ids=list(range(B)), trace=_trace)
    out = np.stack([bkr.results[core]["out"] for core in range(B)])
    if _return_bkr:
        return out, bkr
    return out
